# revision 1
# baseline (speedup 1.0000x reference)
"""Trainium2 Bass kernel for nn_AttentionSampling (sparse window attention block).

Sharding: 8 cores, data-parallel, 1024 windows (half a batch) per core; windows are
independent so there is no cross-core communication. Activations live in a transposed
[d, tokens] layout (host pre-transposes q/k/v) so every projection runs with the
weight stationary and zero on-chip transposes of large tensors. Matmul operands are
bf16 (memory-bound target); attention weights, residuals and LayerNorms stay fp32.

Structure:
- Per-block PE order k-proj -> scores -> v-proj; the windowed-weight
  multiply reads the v-projection PSUM directly via scalar_tensor_tensor
  (fused mul+add accumulation chain).
- Band-masked scores + strided reduction extract the 4 per-window dot
  products as per-partition scalars; f-strided v-projection access
  patterns make the v PSUM rows window-aligned.
- LayerNorm runs in the transposed domain: PE ones-matmuls for token
  sums, rank-1 PE matmul to broadcast mean/rstd across partitions.
- LayerNorm1 batched over 512-token superblocks (N=512 stats matmuls).
- Residual2 fused via scalar_tensor_tensor.
- Single [128,512] broadcast PSUM with two apply rounds; squares tile
  reused as the apply scratch.
"""

import sys
import types

# If BASS_TRACE is set in an environment whose antenv package lacks
# axon_hooks, run_bass_kernel_spmd would crash on import; provide a stub
# (a None hook makes bass_utils skip tracing gracefully).
try:
    import antenv.axon_hooks  # noqa: F401
except ImportError:
    _m = types.ModuleType("antenv.axon_hooks")
    _m.get_axon_ntff_profile_hook = lambda: None
    _m.set_axon_ntff_profile_hook = lambda h: None
    sys.modules["antenv.axon_hooks"] = _m
    try:
        import antenv

        antenv.axon_hooks = _m
    except ImportError:
        pass

import contextlib

import numpy as np

import concourse.bass as bass
import concourse.bacc as bacc_mod
import concourse.mybir as mybir
import concourse.tile as tile
from concourse.bass import ts, ds
from concourse.bass_utils import run_bass_kernel_spmd

FP32 = mybir.dt.float32
FP32R = mybir.dt.float32r
AF = mybir.ActivationFunctionType
OP = mybir.AluOpType

MM_DT = mybir.dt.bfloat16  # matmul operands; attention weights/LN stay fp32

B, SQ, SK, D, F = 4, 2048, 8192, 512, 4
NCORES = 8
WPC = B * SQ // NCORES        # 1024 windows (= tokens) per core
KPC = WPC * F                 # 4096 keys per core
NBLK = WPC // 128             # 8 attention blocks: 128 windows / 512 keys
NSB = WPC // 512              # 2 superblocks of 512 tokens
DT = D // 128                 # 4 d-tiles
EPS = 1e-5

_CACHE = {}


def _emit_ln_T(nc, P, resid_view, sq_tile, stats_sb, out_cb):
    """Transposed LayerNorm over D for a 512-token block.

    resid_view/sq_tile: [128, DT, 512]; sq_tile doubles as apply scratch.
    stats_sb: [1, 1024] (mean | rstd).
    out_cb(dt, src): write normalized+affine output for d-tile dt from src.
    """
    n = 512
    mean = stats_sb[:, :n]
    rstd = stats_sb[:, n : 2 * n]

    nc.vector.tensor_tensor(sq_tile[:], resid_view, resid_view, op=OP.mult)

    st_sum = P["st"].tile([1, 512], FP32, tag="st", name="st_sum")
    for dt in range(DT):
        nc.tensor.matmul(
            st_sum, lhsT=P["ones_col"], rhs=resid_view[:, dt, :],
            start=(dt == 0), stop=(dt == DT - 1),
        )
    nc.scalar.activation(out=mean, in_=st_sum, func=AF.Copy, scale=1.0 / D)

    st_sq = P["st"].tile([1, 512], FP32, tag="st", name="st_sq")
    for dt in range(DT):
        nc.tensor.matmul(
            st_sq, lhsT=P["ones_col"], rhs=sq_tile[:, dt, :],
            start=(dt == 0), stop=(dt == DT - 1),
        )
    e2 = P["small"].tile([1, 512], FP32, tag="e2", name="e2")
    var = P["small"].tile([1, 512], FP32, tag="var", name="var")
    nc.scalar.activation(out=e2, in_=st_sq, func=AF.Copy, scale=1.0 / D)
    nc.vector.tensor_tensor(var, mean, mean, op=OP.mult)
    nc.vector.tensor_tensor(var, e2, var, op=OP.subtract)
    nc.scalar.activation(out=var, in_=var, func=AF.Sqrt, bias=P["eps_t"], scale=1.0)
    nc.vector.reciprocal_approx_fast(out=rstd, in_=var)

    # round 1: subtract broadcast mean (sq_tile becomes the scratch)
    bc = P["bc"].tile([128, 512], FP32, tag="bc", name="bc_mean")
    nc.tensor.matmul(bc, lhsT=P["ones_row"], rhs=mean, start=True, stop=True)
    for dt in range(DT):
        nc.vector.tensor_tensor(sq_tile[:, dt, :], resid_view[:, dt, :], bc, op=OP.subtract)
    # round 2: multiply broadcast rstd (in place), then affine via ACT
    bc2 = P["bc"].tile([128, 512], FP32, tag="bc", name="bc_rstd")
    nc.tensor.matmul(bc2, lhsT=P["ones_row"], rhs=rstd, start=True, stop=True)
    for dt in range(DT):
        nc.vector.tensor_tensor(sq_tile[:, dt, :], sq_tile[:, dt, :], bc2, op=OP.mult)
        out_cb(dt, sq_tile[:, dt, :])


def build_program():
    nc = bacc_mod.Bacc(None, target_bir_lowering=False)

    qT_d = nc.dram_tensor("qT", [D, WPC], MM_DT, kind="ExternalInput")
    kT_d = nc.dram_tensor("kT", [D, KPC], MM_DT, kind="ExternalInput")
    vT_d = nc.dram_tensor("vT", [D, KPC], MM_DT, kind="ExternalInput")
    wq_d = nc.dram_tensor("w_q", [D, D], MM_DT, kind="ExternalInput")
    wk_d = nc.dram_tensor("w_k", [D, D], MM_DT, kind="ExternalInput")
    wv_d = nc.dram_tensor("w_v", [D, D], MM_DT, kind="ExternalInput")
    w1_d = nc.dram_tensor("ffn_w1", [D, D], MM_DT, kind="ExternalInput")
    w2_d = nc.dram_tensor("ffn_w2", [D, D], MM_DT, kind="ExternalInput")
    bq_d = nc.dram_tensor("b_q", [D], FP32, kind="ExternalInput")
    bk_d = nc.dram_tensor("b_k", [D], FP32, kind="ExternalInput")
    bv_d = nc.dram_tensor("b_v", [D], FP32, kind="ExternalInput")
    b1_d = nc.dram_tensor("ffn_b1", [D], FP32, kind="ExternalInput")
    b2_d = nc.dram_tensor("ffn_b2", [D], FP32, kind="ExternalInput")
    g1_d = nc.dram_tensor("ln1_g", [D], FP32, kind="ExternalInput")
    gb1_d = nc.dram_tensor("ln1_b", [D], FP32, kind="ExternalInput")
    g2_d = nc.dram_tensor("ln2_g", [D], FP32, kind="ExternalInput")
    gb2_d = nc.dram_tensor("ln2_b", [D], FP32, kind="ExternalInput")
    mask_d = nc.dram_tensor("cmask", [128, 512], FP32, kind="ExternalInput")
    ident_d = nc.dram_tensor("cident", [128, 128], FP32, kind="ExternalInput")
    outT_d = nc.dram_tensor("outT", [D, WPC], FP32, kind="ExternalOutput")

    qT_t = qT_d.rearrange("(o p) n -> p o n", p=128)
    kT_t = kT_d.rearrange("(o p) n -> p o n", p=128)
    vT_t = vT_d.rearrange("(o p) n -> p o n", p=128)
    outT_t = outT_d.rearrange("(o p) n -> p o n", p=128)

    with tile.TileContext(nc) as tc, contextlib.ExitStack() as ctx:
        singles = ctx.enter_context(tc.tile_pool(name="singles", bufs=1))
        inp = ctx.enter_context(tc.tile_pool(name="inp", bufs=4))
        ktp_p = ctx.enter_context(tc.tile_pool(name="ktp", bufs=1))
        att_p = ctx.enter_context(tc.tile_pool(name="att", bufs=2))
        resid_p = ctx.enter_context(tc.tile_pool(name="resid", bufs=2))
        hT_p = ctx.enter_context(tc.tile_pool(name="hT", bufs=1))
        out_p = ctx.enter_context(tc.tile_pool(name="outp", bufs=2))
        small = ctx.enter_context(tc.tile_pool(name="small", bufs=1))
        ps_proj = ctx.enter_context(tc.tile_pool(name="ps_proj", bufs=3, space="PSUM"))
        ps_vf = ctx.enter_context(tc.tile_pool(name="ps_vf", bufs=2, space="PSUM"))
        ps_tr = ctx.enter_context(tc.tile_pool(name="ps_tr", bufs=1, space="PSUM"))
        ps_st = ctx.enter_context(tc.tile_pool(name="ps_st", bufs=1, space="PSUM"))
        ps_bc = ctx.enter_context(tc.tile_pool(name="ps_bc", bufs=1, space="PSUM"))

        def load_w(d, tg):
            t = singles.tile([128, DT, 512], MM_DT, tag=tg)
            nc.sync.dma_start(out=t, in_=d.rearrange("(o p) n -> p o n", p=128))
            return t

        def load_b(d, tg):
            t = singles.tile([128, DT], FP32, tag=tg)
            nc.sync.dma_start(out=t, in_=d.rearrange("(o p) -> p o", p=128))
            return t

        # issue order matters: Sync issues DMAs in program order, and the PE's
        # first work (q-proj then k-proj of block 0) must not wait behind a
        # dozen constant loads.
        wq_sb = load_w(wq_d, "wq")
        bq_sb = load_b(bq_d, "bq")
        kv0 = []
        for dsrc, t_src in ((kT_d, kT_t), (vT_d, vT_t)):
            t = inp.tile([128, DT, 512], MM_DT, tag="in_t", name="kv0")
            nc.sync.dma_start(out=t, in_=t_src[:, :, ts(0, 512)])
            kv0.append(t)

        wk_sb = load_w(wk_d, "wk")
        bk_sb = load_b(bk_d, "bk")
        wv_sb = load_w(wv_d, "wv")
        mask = singles.tile([128, 512], FP32, tag="mask")
        nc.sync.dma_start(out=mask, in_=mask_d[:, :])
        bv_rep = singles.tile([128, 512], FP32, tag="bv_rep")
        nc.gpsimd.dma_start(
            out=bv_rep, in_=bass.AP(tensor=bv_d, offset=0, ap=[[0, 128], [1, 512]])
        )
        identity = singles.tile([128, 128], FP32, tag="ident")
        nc.sync.dma_start(out=identity, in_=ident_d[:, :])
        g1_sb = load_b(g1_d, "g1")
        gb1_sb = load_b(gb1_d, "gb1")
        ones_col = singles.tile([128, 1], FP32, tag="ones_col")
        nc.gpsimd.memset(ones_col, 1.0)
        ones_row = singles.tile([1, 128], FP32, tag="ones_row")
        nc.gpsimd.memset(ones_row, 1.0)
        eps_t = singles.tile([1, 1], FP32, tag="eps")
        nc.gpsimd.memset(eps_t, EPS)
        late = {}

        def load_late_consts():
            late["w1"] = load_w(w1_d, "w1")
            late["b1"] = load_b(b1_d, "b1")
            late["w2"] = load_w(w2_d, "w2")
            late["b2"] = load_b(b2_d, "b2")
            late["g2"] = load_b(g2_d, "g2")
            late["gb2"] = load_b(gb2_d, "gb2")

        P = {
            "st": ps_st, "bc": ps_bc, "small": small,
            "ones_col": ones_col, "ones_row": ones_row, "eps_t": eps_t,
        }

        qTp = singles.tile([128, DT, WPC], MM_DT, tag="qTp")
        xT = singles.tile([128, DT, WPC], MM_DT, tag="xT")

        def proj_T(w_sb, bias_sb, in_sb, out_sb, out_col0, n):
            for do in range(DT):
                ps = ps_proj.tile([128, 512], FP32, tag="proj_ps", name="proj_ps")
                ps = ps[:, :n]
                for ki in range(DT):
                    nc.tensor.matmul(
                        ps, lhsT=w_sb[:, ki, ts(do, 128)], rhs=in_sb[:, ki, :n],
                        start=(ki == 0), stop=(ki == DT - 1),
                    )
                nc.scalar.activation(
                    out=out_sb[:, do, ds(out_col0, n)], in_=ps, func=AF.Relu,
                    bias=bias_sb[:, do : do + 1], scale=1.0,
                )

        # ---- phase 1: q projection ----
        for blk in range(NSB):
            q_in = inp.tile([128, DT, 512], MM_DT, tag="in_t")
            nc.sync.dma_start(out=q_in, in_=qT_t[:, :, ts(blk, 512)])
            proj_T(wq_sb, bq_sb, q_in, qTp, blk * 512, 512)

        # ---- phase 2: attention ----
        residT = {}  # superblock -> tile [128, DT, 512]

        def emit_front(b):
            if b == 0:
                k_in, v_in = kv0
            else:
                k_in = inp.tile([128, DT, 512], MM_DT, tag="in_t")
                nc.sync.dma_start(out=k_in, in_=kT_t[:, :, ts(b, 512)])
                v_in = inp.tile([128, DT, 512], MM_DT, tag="in_t")
                nc.sync.dma_start(out=v_in, in_=vT_t[:, :, ts(b, 512)])

            kTp = ktp_p.tile([128, DT, 512], MM_DT, tag="kTp")
            proj_T(wk_sb, bk_sb, k_in, kTp, 0, 512)

            sc_ps = ps_proj.tile([128, 512], FP32, tag="proj_ps", name="sc_ps")
            for ki in range(DT):
                nc.tensor.matmul(
                    sc_ps, lhsT=qTp[:, ki, ts(b, 128)], rhs=kTp[:, ki, :],
                    start=(ki == 0), stop=(ki == DT - 1),
                )
            sm = att_p.tile([128, 512], FP32, tag="sm")
            nc.vector.tensor_tensor(sm, sc_ps, mask, op=OP.mult)
            wts = small.tile([128, F], FP32, tag="wts")
            nc.vector.tensor_reduce(
                out=wts, in_=sm.rearrange("p (kw f) -> p f kw", f=F),
                axis=mybir.AxisListType.X, op=OP.add,
            )
            # v projection (f-strided) + fused weighted accumulation from PSUM
            acc = bv_rep
            for f in range(F):
                ps = ps_vf.tile([128, 512], FP32, tag="vf", name="vf_ps")
                for ki in range(DT):
                    nc.tensor.matmul(
                        ps, lhsT=v_in[:, ki, f::4], rhs=wv_sb[:, ki, :],
                        start=(ki == 0), stop=(ki == DT - 1),
                    )
                # final accumulator survives into emit_mid of the NEXT loop
                # iteration — give it its own tag so the next block's chain
                # tiles can't deadlock on its slot (DVE<->PE slot cycle).
                tg = "ao_final" if f == F - 1 else "ao_acc"
                nxt = att_p.tile([128, 512], FP32, tag=tg, name="ao_acc")
                nc.vector.scalar_tensor_tensor(
                    out=nxt, in0=ps, scalar=wts[:, f : f + 1], in1=acc,
                    op0=OP.mult, op1=OP.add,
                )
                acc = nxt
            return acc

        def emit_mid(b, acc):
            sb, col = b // 4, (b % 4) * 128
            if col == 0:
                residT[sb] = resid_p.tile([128, DT, 512], FP32, tag="residT", name="residT")
            r = residT[sb]
            for dt in range(DT):
                ps_t = ps_tr.tile([128, 128], FP32, tag="tr_ps", name="tr_ps")
                nc.tensor.transpose(ps_t, acc[:, ts(dt, 128)], identity)
                nc.vector.tensor_tensor(
                    r[:, dt, ds(col, 128)], ps_t, qTp[:, dt, ts(b, 128)], op=OP.add
                )

        def emit_ln1(sb):
            sq = resid_p.tile([128, DT, 512], FP32, tag="sq1")
            stats = small.tile([1, 1024], FP32, tag="stats1")

            def write_x(dt, src):
                nc.scalar.activation(
                    out=xT[:, dt, ts(sb, 512)], in_=src, func=AF.Identity,
                    bias=gb1_sb[:, dt : dt + 1], scale=g1_sb[:, dt : dt + 1],
                )

            _emit_ln_T(nc, P, residT[sb][:], sq, stats, write_x)

        def emit_ffn(blk):
            hT = hT_p.tile([128, DT, 512], MM_DT, tag="hT")
            for ht in range(DT):
                ps = ps_proj.tile([128, 512], FP32, tag="proj_ps", name="ffn1_ps")
                for ki in range(DT):
                    nc.tensor.matmul(
                        ps, lhsT=late["w1"][:, ki, ts(ht, 128)], rhs=xT[:, ki, ts(blk, 512)],
                        start=(ki == 0), stop=(ki == DT - 1),
                    )
                nc.scalar.activation(
                    out=hT[:, ht, :], in_=ps, func=AF.Relu,
                    bias=late["b1"][:, ht : ht + 1], scale=1.0,
                )
            resid2 = resid_p.tile([128, DT, 512], FP32, tag="resid2")
            for dt in range(DT):
                ps = ps_proj.tile([128, 512], FP32, tag="proj_ps", name="ffn2_ps")
                for hi in range(DT):
                    nc.tensor.matmul(
                        ps, lhsT=late["w2"][:, hi, ts(dt, 128)], rhs=hT[:, hi, :],
                        start=(hi == 0), stop=(hi == DT - 1),
                    )
                nc.vector.scalar_tensor_tensor(
                    out=resid2[:, dt, :], in0=ps, scalar=late["b2"][:, dt : dt + 1],
                    in1=xT[:, dt, ts(blk, 512)], op0=OP.add, op1=OP.add,
                )
            sq2 = hT_p.tile([128, DT, 512], FP32, tag="sq2")
            stats2 = small.tile([1, 1024], FP32, tag="stats2")
            out_sb = out_p.tile([128, DT, 512], FP32, tag="out_sb")

            def write_out(dt, src, out_sb=out_sb):
                nc.scalar.activation(
                    out=out_sb[:, dt, :], in_=src, func=AF.Identity,
                    bias=late["gb2"][:, dt : dt + 1], scale=late["g2"][:, dt : dt + 1],
                )

            _emit_ln_T(nc, P, resid2[:], sq2, stats2, write_out)
            nc.sync.dma_start(out=outT_t[:, :, ts(blk, 512)], in_=out_sb)

        prev = None
        for b in range(NBLK):
            acc = emit_front(b)
            if b == 0:
                load_late_consts()
            if prev is not None:
                emit_mid(b - 1, prev)
            if b == 4:
                emit_ln1(0)
            prev = acc
        emit_mid(NBLK - 1, prev)
        emit_ln1(NSB - 1)
        # FFN(0) is fully ready here (xT cols 0-511 written since block 5);
        # its matmuls fill the PE while LN1(1)'s DVE/ACT chain drains.
        emit_ffn(0)
        emit_ffn(NSB - 1)

    nc.finalize()
    return nc


def kernel(**inputs):
    if "prog" not in _CACHE:
        _CACHE["prog"] = build_program()
    nc = _CACHE["prog"]

    import ml_dtypes

    f32 = lambda x: np.ascontiguousarray(np.asarray(x), dtype=np.float32)
    bf16 = lambda x: np.ascontiguousarray(np.asarray(x, dtype=np.float32).astype(ml_dtypes.bfloat16))
    query, key_, value = f32(inputs["query"]), f32(inputs["key"]), f32(inputs["value"])

    shared = {
        n: f32(inputs[n])
        for n in ("b_q", "b_k", "b_v", "ffn_b1", "ffn_b2",
                  "ln1_g", "ln1_b", "ln2_g", "ln2_b")
    }
    for n in ("w_q", "w_k", "w_v", "ffn_w1", "ffn_w2"):
        shared[n] = bf16(inputs[n])
    p_idx = np.arange(128)[:, None]
    k_idx = np.arange(512)[None, :]
    shared["cmask"] = ((k_idx - 4 * p_idx >= 0) & (k_idx - 4 * p_idx <= 3)).astype(np.float32)
    shared["cident"] = np.eye(128, dtype=np.float32)

    in_maps = []
    for c in range(NCORES):
        bi, half = c // 2, c % 2
        w0 = half * WPC
        m = dict(shared)
        m["qT"] = bf16(query[bi, w0 : w0 + WPC, :].T)
        m["kT"] = bf16(key_[bi, w0 * F : (w0 + WPC) * F, :].T)
        m["vT"] = bf16(value[bi, w0 * F : (w0 + WPC) * F, :].T)
        in_maps.append(m)

    res = run_bass_kernel_spmd(nc, in_maps, core_ids=list(range(NCORES)))
    _CACHE["last_result"] = res
    out = np.empty((B, SQ, D), dtype=np.float32)
    for c in range(NCORES):
        bi, half = c // 2, c % 2
        w0 = half * WPC
        out[bi, w0 : w0 + WPC, :] = res.results[c]["outT"].T
    return out



# revision 15
# speedup vs baseline: 1.2778x; 1.2778x over previous
"""Trainium2 Bass kernel for nn_AttentionSampling (sparse window attention block).

Sharding: 8 cores, data-parallel, 1024 windows (half a batch) per core; windows are
independent so there is no cross-core communication. q/k live in a transposed
[d, tokens] layout (host pre-transposes) so projections run weight-stationary;
v stays in natural [keys, d] layout so the banded attention aggregation can run
as PE matmuls against the masked score matrix.

Structure (per 128-window / 512-key block):
- k-proj (N=512 bf16 matmuls) -> scores computed directly TRANSPOSED
  ([keys, windows], 16 N=128 matmuls) -> DVE band-mask multiply produces the
  sparse weight matrix W [512 keys, 128 windows] in bf16.
- Attention output via aggregate-then-project: avT = v_nat.T-contracted with W
  (16 N=128 MMs), then aoT = wv.T @ avT + bv x colsum(W) (20 N=128 MMs).
  4x fewer v-projection MACs than project-then-aggregate, no PE transposes,
  and the residual add is a single 3D DVE op.
- LayerNorm stats (ones-matmul token sums) and mean/rstd broadcasts run as
  FP32R matmuls: full fp32 data, 1 cycle/row at N=512 (fp32 is 4).
- Software pipeline: iteration b emits k-proj(b), scoresT(b-1), v-agg(b-2),
  aoT(b-3) so each PE group's ACT/DVE dependencies are >=1 block old.
- PE warmup matmuls during the initial DMA fill keep the HAM clock gate from
  running the first real matmuls at 1.2 GHz.
- LN2 apply + output DMA chunked per d-tile to shrink the serial tail.
"""

import sys
import types

# If BASS_TRACE is set in an environment whose antenv package lacks
# axon_hooks, run_bass_kernel_spmd would crash on import; provide a stub
# (a None hook makes bass_utils skip tracing gracefully).
try:
    import antenv.axon_hooks  # noqa: F401
except ImportError:
    _m = types.ModuleType("antenv.axon_hooks")
    _m.get_axon_ntff_profile_hook = lambda: None
    _m.set_axon_ntff_profile_hook = lambda h: None
    sys.modules["antenv.axon_hooks"] = _m
    try:
        import antenv

        antenv.axon_hooks = _m
    except ImportError:
        pass

import contextlib

import numpy as np

import concourse.bass as bass
import concourse.bacc as bacc_mod
import concourse.mybir as mybir
import concourse.tile as tile
from concourse.bass import ts, ds
from concourse.bass_utils import run_bass_kernel_spmd

FP32 = mybir.dt.float32
FP16 = mybir.dt.float16
AF = mybir.ActivationFunctionType
OP = mybir.AluOpType

MM_DT = mybir.dt.bfloat16  # matmul operands
# Residual stream and LN stats run in fp16 (11-bit mantissa, full-rate PE
# matmuls for the stats/broadcast ones-products). Squares use bf16 for range
# (resid^2 reaches ~8e5 > fp16 max); their quantization only perturbs the
# variance by ~1e-4 relative. |mu|/sigma <= 0.17 here so fp16 mean is safe.

B, SQ, SK, D, F = 4, 2048, 8192, 512, 4
NCORES = 8
WPC = B * SQ // NCORES        # 1024 windows (= tokens) per core
KPC = WPC * F                 # 4096 keys per core
NBLK = WPC // 128             # 8 attention blocks: 128 windows / 512 keys
NSB = WPC // 512              # 2 superblocks of 512 tokens
DT = D // 128                 # 4 d-tiles
KC = 4                        # key chunks per block (512 keys / 128)
EPS = 1e-5
N_WARMUP = 8                  # PE warmup matmuls during initial DMA fill

_CACHE = {}


def build_program():
    nc = bacc_mod.Bacc(None, target_bir_lowering=False)

    qT_d = nc.dram_tensor("qT", [D, WPC], MM_DT, kind="ExternalInput")
    kT_d = nc.dram_tensor("kT", [D, KPC], MM_DT, kind="ExternalInput")
    vN_d = nc.dram_tensor("vN", [KPC, D], MM_DT, kind="ExternalInput")
    wq_d = nc.dram_tensor("w_q", [D, D], MM_DT, kind="ExternalInput")
    wk_d = nc.dram_tensor("w_k", [D, D], MM_DT, kind="ExternalInput")
    wv_d = nc.dram_tensor("w_v", [D, D], MM_DT, kind="ExternalInput")
    w1_d = nc.dram_tensor("ffn_w1", [D, D], MM_DT, kind="ExternalInput")
    w2_d = nc.dram_tensor("ffn_w2", [D, D], MM_DT, kind="ExternalInput")
    bq_d = nc.dram_tensor("b_q", [D], FP32, kind="ExternalInput")
    bk_d = nc.dram_tensor("b_k", [D], FP32, kind="ExternalInput")
    bvr_d = nc.dram_tensor("bv_row", [1, D], MM_DT, kind="ExternalInput")
    b1_d = nc.dram_tensor("ffn_b1", [D], FP32, kind="ExternalInput")
    b2_d = nc.dram_tensor("ffn_b2", [D], FP32, kind="ExternalInput")
    g1_d = nc.dram_tensor("ln1_g", [D], FP32, kind="ExternalInput")
    gb1_d = nc.dram_tensor("ln1_b", [D], FP32, kind="ExternalInput")
    g2_d = nc.dram_tensor("ln2_g", [D], FP32, kind="ExternalInput")
    gb2_d = nc.dram_tensor("ln2_b", [D], FP32, kind="ExternalInput")
    maskT_d = nc.dram_tensor("maskT", [128, KC, 128], FP32, kind="ExternalInput")
    outT_d = nc.dram_tensor("outT", [D, WPC], FP32, kind="ExternalOutput")

    qT_t = qT_d.rearrange("(o p) n -> p o n", p=128)
    kT_t = kT_d.rearrange("(o p) n -> p o n", p=128)
    vN_t = vN_d.rearrange("(nb kc p) d -> p nb kc d", p=128, kc=KC)
    outT_t = outT_d.rearrange("(o p) n -> p o n", p=128)

    with tile.TileContext(nc) as tc, contextlib.ExitStack() as ctx:
        # PSUM budget is 8 banks x 2KB: proj(2) + sc(1) + av(1) + ao(1) +
        # stats/bc shared tag(2) + srow(1) = 8.
        singles = ctx.enter_context(tc.tile_pool(name="singles", bufs=1))
        qin_p = ctx.enter_context(tc.tile_pool(name="qin", bufs=2))
        kin_p = ctx.enter_context(tc.tile_pool(name="kin", bufs=3))
        vin_p = ctx.enter_context(tc.tile_pool(name="vin", bufs=5))
        ktp_p = ctx.enter_context(tc.tile_pool(name="ktp", bufs=2))
        w_p = ctx.enter_context(tc.tile_pool(name="wsb", bufs=2))
        av_p = ctx.enter_context(tc.tile_pool(name="avsb", bufs=2))
        resid_p = ctx.enter_context(tc.tile_pool(name="resid", bufs=2))
        hT_p = ctx.enter_context(tc.tile_pool(name="hT", bufs=2))
        out_p = ctx.enter_context(tc.tile_pool(name="outp", bufs=2))
        small = ctx.enter_context(tc.tile_pool(name="small", bufs=1))
        ps_proj = ctx.enter_context(tc.tile_pool(name="ps_proj", bufs=2, space="PSUM"))
        ps_sc = ctx.enter_context(tc.tile_pool(name="ps_sc", bufs=1, space="PSUM"))
        ps_av = ctx.enter_context(tc.tile_pool(name="ps_av", bufs=1, space="PSUM"))
        ps_ao = ctx.enter_context(tc.tile_pool(name="ps_ao", bufs=1, space="PSUM"))
        ps_misc = ctx.enter_context(tc.tile_pool(name="ps_misc", bufs=2, space="PSUM"))

        def load_w(d, tg):
            t = singles.tile([128, DT, 512], MM_DT, tag=tg)
            nc.sync.dma_start(out=t, in_=d.rearrange("(o p) n -> p o n", p=128))
            return t

        def load_b(d, tg):
            t = singles.tile([128, DT], FP32, tag=tg)
            nc.sync.dma_start(out=t, in_=d.rearrange("(o p) -> p o", p=128))
            return t

        # Warmup scratch needs no DMA: memset, then matmuls on it below.
        warm_sb = singles.tile([128, 512], MM_DT, tag="warm")
        nc.gpsimd.memset(warm_sb, 0.001)
        ones_colh = singles.tile([128, 1], FP16, tag="ones_colh")
        nc.gpsimd.memset(ones_colh, 1.0)
        ones_colb = singles.tile([128, 1], MM_DT, tag="ones_colb")
        nc.gpsimd.memset(ones_colb, 1.0)
        ones_rowh = singles.tile([1, 128], FP16, tag="ones_rowh")
        nc.gpsimd.memset(ones_rowh, 1.0)
        eps_t = singles.tile([1, 1], FP32, tag="eps")
        nc.gpsimd.memset(eps_t, EPS)

        # DMA issue order = need order: q path first, then block-0/1 k+v,
        # then the rest of the constants.
        wq_sb = load_w(wq_d, "wq")
        bq_sb = load_b(bq_d, "bq")
        q_in = []
        for sb in range(NSB):
            t = qin_p.tile([128, DT, 512], MM_DT, tag="q_in", name="q_in")
            nc.sync.dma_start(out=t, in_=qT_t[:, :, ts(sb, 512)])
            q_in.append(t)
        wk_sb = load_w(wk_d, "wk")
        bk_sb = load_b(bk_d, "bk")

        kv_tiles = {}

        def prefetch_kv(b):
            if b >= NBLK:
                return
            k_t = kin_p.tile([128, DT, 512], MM_DT, tag="k_in", name="k_in")
            nc.sync.dma_start(out=k_t, in_=kT_t[:, :, ts(b, 512)])
            v_t = vin_p.tile([128, KC, 512], MM_DT, tag="v_in", name="v_in")
            nc.sync.dma_start(out=v_t, in_=vN_t[:, b, :, :])
            kv_tiles[b] = (k_t, v_t)

        prefetch_kv(0)
        wv_sb = load_w(wv_d, "wv")
        prefetch_kv(1)
        maskT = singles.tile([128, KC, 128], FP32, tag="maskT")
        nc.sync.dma_start(out=maskT, in_=maskT_d[:, :, :])
        bv_row = singles.tile([1, 512], MM_DT, tag="bv_row")
        nc.sync.dma_start(out=bv_row, in_=bvr_d[:, :])
        g1_sb = load_b(g1_d, "g1")
        gb1_sb = load_b(gb1_d, "gb1")

        late = {}

        def load_late_consts():
            late["w1"] = load_w(w1_d, "w1")
            late["b1"] = load_b(b1_d, "b1")
            late["w2"] = load_w(w2_d, "w2")
            late["b2"] = load_b(b2_d, "b2")
            late["g2"] = load_b(g2_d, "g2")
            late["gb2"] = load_b(gb2_d, "gb2")

        # ---- PE warmup: trip the HAM clock gate while DMAs fill ----
        for i in range(N_WARMUP):
            wps = ps_proj.tile([128, 512], FP32, tag="proj_ps", name="warm_ps")
            nc.tensor.matmul(wps, lhsT=warm_sb[:, :128], rhs=warm_sb,
                             start=True, stop=True)

        qTp = singles.tile([128, DT, WPC], MM_DT, tag="qTp")
        xT = singles.tile([128, DT, WPC], MM_DT, tag="xT")

        def proj_T(w_sb, bias_sb, in_sb, out_sb, out_col0, n):
            for do in range(DT):
                ps = ps_proj.tile([128, 512], FP32, tag="proj_ps", name="proj_ps")
                ps = ps[:, :n]
                for ki in range(DT):
                    nc.tensor.matmul(
                        ps, lhsT=w_sb[:, ki, ts(do, 128)], rhs=in_sb[:, ki, :n],
                        start=(ki == 0), stop=(ki == DT - 1),
                    )
                nc.scalar.activation(
                    out=out_sb[:, do, ds(out_col0, n)], in_=ps, func=AF.Relu,
                    bias=bias_sb[:, do : do + 1], scale=1.0,
                )

        # ---- phase 1: q projection ----
        for sb in range(NSB):
            proj_T(wq_sb, bq_sb, q_in[sb], qTp, sb * 512, 512)

        # ---- phase 2: attention, software-pipelined ----
        residT = {}  # superblock -> tile [128, DT, 512]
        kTp = {}     # block -> k-projection tile
        W_sb = {}    # block -> masked scoresT (the banded weight matrix)
        avT = {}     # block -> aggregated v tile
        srow = {}    # block -> [1,128] bf16 colsums of W

        def emit_kproj(b):
            k_t, _ = kv_tiles[b]
            kp = ktp_p.tile([128, DT, 512], MM_DT, tag="kTp", name="kTp")
            proj_T(wk_sb, bk_sb, k_t, kp, 0, 512)
            kTp[b] = kp

        def emit_scores(b):
            # scT[k, w] = sum_d kTp[d, k] * qTp[d, w] for this block's keys
            sc_ps = ps_sc.tile([128, KC, 128], FP32, tag="sc_ps", name="sc_ps")
            for kc in range(KC):
                for ki in range(DT):
                    nc.tensor.matmul(
                        sc_ps[:, kc, :],
                        lhsT=kTp[b][:, ki, ts(kc, 128)],
                        rhs=qTp[:, ki, ts(b, 128)],
                        start=(ki == 0), stop=(ki == DT - 1),
                    )
            del kTp[b]
            # band mask -> sparse weight matrix W (bf16, zero off-band)
            w_t = w_p.tile([128, KC, 128], MM_DT, tag="W", name="W")
            nc.vector.tensor_tensor(w_t[:], sc_ps[:], maskT[:], op=OP.mult)
            W_sb[b] = w_t

        def emit_vagg(b):
            _, v_t = kv_tiles.pop(b)
            w_t = W_sb[b]
            av_ps = ps_av.tile([128, DT, 128], FP32, tag="av_ps", name="av_ps")
            for dc in range(DT):
                for kc in range(KC):
                    nc.tensor.matmul(
                        av_ps[:, dc, :],
                        lhsT=v_t[:, kc, ts(dc, 128)],
                        rhs=w_t[:, kc, :],
                        start=(kc == 0), stop=(kc == KC - 1),
                    )
            # srow[w] = sum_k W[k, w]  (for the bias term)
            sr_ps = ps_misc.tile([1, 128], FP32, tag="sr_ps", name="sr_ps", bufs=1)
            for kc in range(KC):
                nc.tensor.matmul(
                    sr_ps, lhsT=ones_colb, rhs=w_t[:, kc, :],
                    start=(kc == 0), stop=(kc == KC - 1),
                )
            av_t = av_p.tile([128, DT, 128], MM_DT, tag="avT", name="avT")
            nc.scalar.activation(out=av_t[:], in_=av_ps[:], func=AF.Copy, scale=1.0)
            sr_t = small.tile([1, 128], MM_DT, tag="srow", name="srow", bufs=2)
            nc.scalar.activation(out=sr_t, in_=sr_ps, func=AF.Copy, scale=1.0)
            avT[b] = av_t
            srow[b] = sr_t
            del W_sb[b]

        def emit_aoproj(b):
            sb, col = b // 4, (b % 4) * 128
            if col == 0:
                residT[sb] = resid_p.tile([128, DT, 512], FP16, tag="residT", name="residT")
            av_t, sr_t = avT.pop(b), srow.pop(b)
            ao_ps = ps_ao.tile([128, DT, 128], FP32, tag="ao_ps", name="ao_ps")
            for do in range(DT):
                for ki in range(DT):
                    nc.tensor.matmul(
                        ao_ps[:, do, :],
                        lhsT=wv_sb[:, ki, ts(do, 128)],
                        rhs=av_t[:, ki, :],
                        start=(ki == 0), stop=False,
                    )
                nc.tensor.matmul(
                    ao_ps[:, do, :],
                    lhsT=bv_row[:, ts(do, 128)], rhs=sr_t,
                    start=False, stop=True,
                )
            # residual: residT[:, :, col:col+128] = aoT + qTp_block  (one 3D op)
            nc.vector.tensor_tensor(
                residT[sb][:, :, ds(col, 128)], ao_ps[:],
                qTp[:, :, ts(b, 128)], op=OP.add,
            )

        def emit_ln(resid_t, g_sb, gb_sb, out_cb, out_dt_chunked=False):
            """Transposed LayerNorm over D for one 512-token superblock.

            resid_t: [128, DT, 512] fp16. out_cb(dt) -> output AP for d-tile dt.
            Stats run as fp16/bf16 ones-matmuls (1 cycle/row), broadcasts as
            fp16 rank-1 matmuls; PSUM accumulation is fp32 throughout.
            """
            sq = hT_p.tile([128, DT, 512], MM_DT, tag="sq", name="sq")
            nc.vector.tensor_tensor(sq[:], resid_t[:], resid_t[:], op=OP.mult)
            st_sum = ps_misc.tile([1, 512], FP32, tag="st", name="st_sum")
            for dt in range(DT):
                nc.tensor.matmul(
                    st_sum, lhsT=ones_colh, rhs=resid_t[:, dt, :],
                    start=(dt == 0), stop=(dt == DT - 1),
                )
            st_sq = ps_misc.tile([1, 512], FP32, tag="st", name="st_sq")
            for dt in range(DT):
                nc.tensor.matmul(
                    st_sq, lhsT=ones_colb, rhs=sq[:, dt, :],
                    start=(dt == 0), stop=(dt == DT - 1),
                )
            mean = small.tile([1, 512], FP16, tag="mean", name="mean")
            nc.scalar.activation(out=mean, in_=st_sum, func=AF.Copy, scale=1.0 / D)
            e2 = small.tile([1, 512], FP32, tag="e2", name="e2")
            var = small.tile([1, 512], FP32, tag="var", name="var")
            nc.scalar.activation(out=e2, in_=st_sq, func=AF.Copy, scale=1.0 / D)
            nc.vector.tensor_tensor(var, mean, mean, op=OP.mult)
            nc.vector.tensor_tensor(var, e2, var, op=OP.subtract)
            nc.scalar.activation(out=var, in_=var, func=AF.Sqrt, bias=eps_t, scale=1.0)
            rstd32 = small.tile([1, 512], FP32, tag="rstd32", name="rstd32")
            nc.vector.reciprocal_approx_fast(out=rstd32, in_=var)
            rstd = small.tile([1, 512], FP16, tag="rstd", name="rstd")
            nc.scalar.activation(out=rstd, in_=rstd32, func=AF.Copy, scale=1.0)

            # bc tiles share the "st" tag/banks: st_sum/st_sq are consumed by
            # the ACT copies above before these are written.
            bcm = ps_misc.tile([128, 512], FP32, tag="st", name="bcm")
            nc.tensor.matmul(bcm, lhsT=ones_rowh, rhs=mean, start=True, stop=True)
            bcr = ps_misc.tile([128, 512], FP32, tag="st", name="bcr")
            nc.tensor.matmul(bcr, lhsT=ones_rowh, rhs=rstd, start=True, stop=True)

            # t = (resid - mu) * rstd (fp16 scratch; centered so quantization
            # is relative to the normalized scale); out = t*g + b (ACT)
            tmp = hT_p.tile([128, DT, 512], FP16, tag="tscr", name="tscr")
            for dt in range(DT):
                nc.vector.tensor_tensor(tmp[:, dt, :], resid_t[:, dt, :], bcm, op=OP.subtract)
                nc.vector.tensor_tensor(tmp[:, dt, :], tmp[:, dt, :], bcr, op=OP.mult)
                nc.scalar.activation(
                    out=out_cb(dt), in_=tmp[:, dt, :], func=AF.Identity,
                    bias=gb_sb[:, dt : dt + 1], scale=g_sb[:, dt : dt + 1],
                )
                if out_dt_chunked:
                    out_dt_chunked(dt)

        def emit_ln1(sb):
            emit_ln(residT[sb], g1_sb, gb1_sb,
                    lambda dt: xT[:, dt, ts(sb, 512)])

        def emit_ffn1(sb):
            hT = hT_p.tile([128, DT, 512], MM_DT, tag="hT", name="hT")
            for ht in range(DT):
                ps = ps_proj.tile([128, 512], FP32, tag="proj_ps", name="ffn1_ps")
                for ki in range(DT):
                    nc.tensor.matmul(
                        ps, lhsT=late["w1"][:, ki, ts(ht, 128)], rhs=xT[:, ki, ts(sb, 512)],
                        start=(ki == 0), stop=(ki == DT - 1),
                    )
                nc.scalar.activation(
                    out=hT[:, ht, :], in_=ps, func=AF.Relu,
                    bias=late["b1"][:, ht : ht + 1], scale=1.0,
                )
            return hT

        def emit_ffn2(sb, hT):
            resid2 = resid_p.tile([128, DT, 512], FP16, tag="resid2", name="resid2")
            for dt in range(DT):
                ps = ps_proj.tile([128, 512], FP32, tag="proj_ps", name="ffn2_ps")
                for hi in range(DT):
                    nc.tensor.matmul(
                        ps, lhsT=late["w2"][:, hi, ts(dt, 128)], rhs=hT[:, hi, :],
                        start=(hi == 0), stop=(hi == DT - 1),
                    )
                nc.vector.scalar_tensor_tensor(
                    out=resid2[:, dt, :], in0=ps, scalar=late["b2"][:, dt : dt + 1],
                    in1=xT[:, dt, ts(sb, 512)], op0=OP.add, op1=OP.add,
                )
            return resid2

        def emit_ln2_out(sb, resid2):
            out_sb = out_p.tile([128, DT, 512], FP32, tag="out_sb", name="out_sb")

            def dma_dt(dt):
                nc.sync.dma_start(
                    out=outT_t[:, dt, ts(sb, 512)], in_=out_sb[:, dt, :]
                )

            emit_ln(resid2, late["g2"], late["gb2"],
                    lambda dt: out_sb[:, dt, :], out_dt_chunked=dma_dt)

        # pipeline: k-proj(b), scoresT(b-1), v-agg(b-2), aoT(b-3)
        for b in range(NBLK + 3):
            if b < NBLK:
                emit_kproj(b)
            if b == 0:
                prefetch_kv(2)
                load_late_consts()
            elif b < NBLK:
                prefetch_kv(b + 2)
            if 1 <= b <= NBLK:
                emit_scores(b - 1)
            if 2 <= b <= NBLK + 1:
                emit_vagg(b - 2)
            if 3 <= b <= NBLK + 2:
                emit_aoproj(b - 3)
            if b - 3 == NSB * 4 - 5:  # blocks 0..3 residuals complete
                emit_ln1(0)

        emit_ln1(NSB - 1)
        hT0 = emit_ffn1(0)
        r20 = emit_ffn2(0, hT0)
        hT1 = emit_ffn1(1)
        emit_ln2_out(0, r20)
        r21 = emit_ffn2(1, hT1)
        emit_ln2_out(1, r21)

    nc.finalize()
    return nc


def kernel(**inputs):
    if "prog" not in _CACHE:
        _CACHE["prog"] = build_program()
    nc = _CACHE["prog"]

    import ml_dtypes

    f32 = lambda x: np.ascontiguousarray(np.asarray(x), dtype=np.float32)
    bf16 = lambda x: np.ascontiguousarray(np.asarray(x, dtype=np.float32).astype(ml_dtypes.bfloat16))
    query, key_, value = f32(inputs["query"]), f32(inputs["key"]), f32(inputs["value"])

    shared = {
        n: f32(inputs[n])
        for n in ("b_q", "b_k", "ffn_b1", "ffn_b2",
                  "ln1_g", "ln1_b", "ln2_g", "ln2_b")
    }
    for n in ("w_q", "w_k", "w_v", "ffn_w1", "ffn_w2"):
        shared[n] = bf16(inputs[n])
    shared["bv_row"] = bf16(np.asarray(inputs["b_v"], dtype=np.float32).reshape(1, D))
    # maskT[p, kc, w] = 1 where key (kc*128+p) belongs to window w of the block
    p_idx = np.arange(128)[:, None, None]
    kc_idx = np.arange(KC)[None, :, None]
    w_idx = np.arange(128)[None, None, :]
    shared["maskT"] = (w_idx == kc_idx * 32 + p_idx // 4).astype(np.float32)

    in_maps = []
    for c in range(NCORES):
        bi, half = c // 2, c % 2
        w0 = half * WPC
        m = dict(shared)
        m["qT"] = bf16(query[bi, w0 : w0 + WPC, :].T)
        m["kT"] = bf16(key_[bi, w0 * F : (w0 + WPC) * F, :].T)
        m["vN"] = bf16(value[bi, w0 * F : (w0 + WPC) * F, :])
        in_maps.append(m)

    res = run_bass_kernel_spmd(nc, in_maps, core_ids=list(range(NCORES)))
    _CACHE["last_result"] = res
    out = np.empty((B, SQ, D), dtype=np.float32)
    for c in range(NCORES):
        bi, half = c // 2, c % 2
        w0 = half * WPC
        out[bi, w0 : w0 + WPC, :] = res.results[c]["outT"].T
    return out


# revision 35
# speedup vs baseline: 1.2843x; 1.0050x over previous
"""Trainium2 Bass kernel for nn_AttentionSampling (sparse window attention block).

Sharding: 8 cores, data-parallel, 1024 windows (half a batch) per core; windows are
independent so there is no cross-core communication. q/k live in a transposed
[d, tokens] layout (host pre-transposes) so projections run weight-stationary;
v stays in natural [keys, d] layout so the banded attention aggregation can run
as PE matmuls against the masked score matrix.

Structure (per 128-window / 512-key block):
- k-proj (N=512 bf16 matmuls) -> scores computed directly TRANSPOSED
  ([keys, windows], 16 N=128 matmuls) -> DVE band-mask multiply produces the
  sparse weight matrix W [512 keys, 128 windows] in bf16.
- Attention output via aggregate-then-project: avT = v_nat.T-contracted with W
  (16 N=128 MMs), then aoT = wv.T @ avT + bv x colsum(W) (20 N=128 MMs).
  4x fewer v-projection MACs than project-then-aggregate, no PE transposes,
  and the residual add is a single 3D DVE op.
- LayerNorm stats (ones-matmul token sums) and mean/rstd broadcasts run as
  FP32R matmuls: full fp32 data, 1 cycle/row at N=512 (fp32 is 4).
- Software pipeline: iteration b emits k-proj(b), scoresT(b-1), v-agg(b-2),
  aoT(b-3) so each PE group's ACT/DVE dependencies are >=1 block old.
- PE warmup matmuls during the initial DMA fill keep the HAM clock gate from
  running the first real matmuls at 1.2 GHz.
- LN2 apply + output DMA chunked per d-tile to shrink the serial tail.
"""

import sys
import types

# If BASS_TRACE is set in an environment whose antenv package lacks
# axon_hooks, run_bass_kernel_spmd would crash on import; provide a stub
# (a None hook makes bass_utils skip tracing gracefully).
try:
    import antenv.axon_hooks  # noqa: F401
except ImportError:
    _m = types.ModuleType("antenv.axon_hooks")
    _m.get_axon_ntff_profile_hook = lambda: None
    _m.set_axon_ntff_profile_hook = lambda h: None
    sys.modules["antenv.axon_hooks"] = _m
    try:
        import antenv

        antenv.axon_hooks = _m
    except ImportError:
        pass

import contextlib

import numpy as np

import concourse.bass as bass
import concourse.bacc as bacc_mod
import concourse.mybir as mybir
import concourse.tile as tile
from concourse.bass import ts, ds
from concourse.bass_utils import run_bass_kernel_spmd

FP32 = mybir.dt.float32
FP16 = mybir.dt.float16
AF = mybir.ActivationFunctionType
OP = mybir.AluOpType

MM_DT = mybir.dt.bfloat16  # matmul operands
# The residual stream and LN stats run in bf16 (fp16 matmuls measured at HALF
# the bf16 rate on TRN2 HW despite the cost model saying otherwise; bf16
# element quantization only costs ~1e-2 worst-element relative error vs the
# 2e-2 gate). PSUM accumulation is fp32. rstd and the centered apply scratch
# stay fp16 (11-bit mantissa) since they multiply the output directly.

B, SQ, SK, D, F = 4, 2048, 8192, 512, 4
NCORES = 8
WPC = B * SQ // NCORES        # 1024 windows (= tokens) per core
KPC = WPC * F                 # 4096 keys per core
NBLK = WPC // 128             # 8 attention blocks: 128 windows / 512 keys
NSB = WPC // 512              # 2 superblocks of 512 tokens
DT = D // 128                 # 4 d-tiles
KC = 4                        # key chunks per block (512 keys / 128)
EPS = 1e-5
N_WARMUP = 10                 # PE warmup matmuls during initial DMA fill

_CACHE = {}


def build_program(use_vbias=True, affine1=True, affine2=True):
    nc = bacc_mod.Bacc(None, target_bir_lowering=False)

    qT_d = nc.dram_tensor("qT", [D, WPC], MM_DT, kind="ExternalInput")
    kT_d = nc.dram_tensor("kT", [D, KPC], MM_DT, kind="ExternalInput")
    vN_d = nc.dram_tensor("vN", [KPC, D], MM_DT, kind="ExternalInput")
    wq_d = nc.dram_tensor("w_q", [D, D], MM_DT, kind="ExternalInput")
    wk_d = nc.dram_tensor("w_k", [D, D], MM_DT, kind="ExternalInput")
    wv_d = nc.dram_tensor("w_v", [D, D], MM_DT, kind="ExternalInput")
    w1_d = nc.dram_tensor("ffn_w1", [D, D], MM_DT, kind="ExternalInput")
    w2_d = nc.dram_tensor("ffn_w2", [D, D], MM_DT, kind="ExternalInput")
    bq_d = nc.dram_tensor("b_q", [D], FP32, kind="ExternalInput")
    bk_d = nc.dram_tensor("b_k", [D], FP32, kind="ExternalInput")
    bvr_d = nc.dram_tensor("bv_row", [1, D], MM_DT, kind="ExternalInput")
    b1_d = nc.dram_tensor("ffn_b1", [D], FP32, kind="ExternalInput")
    b2_d = nc.dram_tensor("ffn_b2", [D], FP32, kind="ExternalInput")
    g1_d = nc.dram_tensor("ln1_g", [D], FP32, kind="ExternalInput")
    gb1_d = nc.dram_tensor("ln1_b", [D], FP32, kind="ExternalInput")
    g2_d = nc.dram_tensor("ln2_g", [D], FP32, kind="ExternalInput")
    gb2_d = nc.dram_tensor("ln2_b", [D], FP32, kind="ExternalInput")
    maskT_d = nc.dram_tensor("maskT", [128, KC, 128], FP32, kind="ExternalInput")
    outT_d = nc.dram_tensor("outT", [D, WPC], FP32, kind="ExternalOutput")

    qT_t = qT_d.rearrange("(o p) n -> p o n", p=128)
    kT_t = kT_d.rearrange("(o p) n -> p o n", p=128)
    vN_t = vN_d.rearrange("(nb kc p) d -> p nb kc d", p=128, kc=KC)
    outT_t = outT_d.rearrange("(o p) n -> p o n", p=128)

    with tile.TileContext(nc) as tc, contextlib.ExitStack() as ctx:
        # PSUM budget is 8 banks x 2KB: proj(2) + sc(1) + av(1) + ao(1) +
        # stats/bc shared tag(2) + srow(1) = 8.
        singles = ctx.enter_context(tc.tile_pool(name="singles", bufs=1))
        qin_p = ctx.enter_context(tc.tile_pool(name="qin", bufs=2))
        kin_p = ctx.enter_context(tc.tile_pool(name="kin", bufs=3))
        vin_p = ctx.enter_context(tc.tile_pool(name="vin", bufs=5))
        ktp_p = ctx.enter_context(tc.tile_pool(name="ktp", bufs=2))
        w_p = ctx.enter_context(tc.tile_pool(name="wsb", bufs=2))
        av_p = ctx.enter_context(tc.tile_pool(name="avsb", bufs=2))
        resid_p = ctx.enter_context(tc.tile_pool(name="resid", bufs=2))
        hT_p = ctx.enter_context(tc.tile_pool(name="hT", bufs=2))
        out_p = ctx.enter_context(tc.tile_pool(name="outp", bufs=2))
        small = ctx.enter_context(tc.tile_pool(name="small", bufs=1))
        ps_proj = ctx.enter_context(tc.tile_pool(name="ps_proj", bufs=2, space="PSUM"))
        ps_sc = ctx.enter_context(tc.tile_pool(name="ps_sc", bufs=1, space="PSUM"))
        ps_av = ctx.enter_context(tc.tile_pool(name="ps_av", bufs=1, space="PSUM"))
        ps_ao = ctx.enter_context(
            tc.tile_pool(name="ps_ao", bufs=1 if use_vbias else 2, space="PSUM"))
        ps_misc = ctx.enter_context(tc.tile_pool(name="ps_misc", bufs=2, space="PSUM"))

        def load_w(d, tg):
            t = singles.tile([128, DT, 512], MM_DT, tag=tg)
            nc.sync.dma_start(out=t, in_=d.rearrange("(o p) n -> p o n", p=128))
            return t

        def load_b(d, tg):
            t = singles.tile([128, DT], FP32, tag=tg)
            nc.sync.dma_start(out=t, in_=d.rearrange("(o p) -> p o", p=128))
            return t

        # Warmup scratch needs no DMA: memset, then matmuls on it below.
        warm_sb = singles.tile([128, 512], MM_DT, tag="warm")
        nc.vector.memset(warm_sb, 0.001)
        ones_colb = singles.tile([128, 1], MM_DT, tag="ones_colb")
        nc.gpsimd.memset(ones_colb, 1.0)
        ones_rowb = singles.tile([1, 128], MM_DT, tag="ones_rowb")
        nc.gpsimd.memset(ones_rowb, 1.0)
        ones_rowh = singles.tile([1, 128], FP16, tag="ones_rowh")
        nc.gpsimd.memset(ones_rowh, 1.0)

        kv_tiles = {}

        def prefetch_kv(b, k_engine=None):
            if b >= NBLK:
                return
            k_t = kin_p.tile([128, DT, 512], MM_DT, tag="k_in", name="k_in")
            (k_engine or nc.sync).dma_start(out=k_t, in_=kT_t[:, :, ts(b, 512)])
            v_t = vin_p.tile([128, KC, 512], MM_DT, tag="v_in", name="v_in")
            nc.gpsimd.dma_start(out=v_t, in_=vN_t[:, b, :, :])
            kv_tiles[b] = (k_t, v_t)

        # DMA issue order = need order: wq+q0 first (PE's first work), then
        # wk+k0, q1, k1; v loads issue from the gpsimd queue in parallel.
        wq_sb = load_w(wq_d, "wq")
        q_in = []
        for sb in range(NSB):
            t = qin_p.tile([128, DT, 512], MM_DT, tag="q_in", name="q_in")
            q_in.append(t)
        nc.sync.dma_start(out=q_in[0], in_=qT_t[:, :, ts(0, 512)])
        wk_sb = load_w(wk_d, "wk")
        prefetch_kv(0)
        nc.sync.dma_start(out=q_in[1], in_=qT_t[:, :, ts(1, 512)])
        bq_sb = load_b(bq_d, "bq")
        bk_sb = load_b(bk_d, "bk")
        prefetch_kv(1)
        wv_sb = load_w(wv_d, "wv")
        maskT = singles.tile([128, KC, 128], FP32, tag="maskT")
        nc.sync.dma_start(out=maskT, in_=maskT_d[:, :, :])
        bv_row = None
        if use_vbias:
            bv_row = singles.tile([1, 512], MM_DT, tag="bv_row")
            nc.sync.dma_start(out=bv_row, in_=bvr_d[:, :])
        g1_sb = load_b(g1_d, "g1")
        gb1_sb = load_b(gb1_d, "gb1")

        late = {}

        def load_late_consts():
            late["w1"] = load_w(w1_d, "w1")
            late["b1"] = load_b(b1_d, "b1")
            late["w2"] = load_w(w2_d, "w2")
            late["b2"] = load_b(b2_d, "b2")
            late["g2"] = load_b(g2_d, "g2")
            late["gb2"] = load_b(gb2_d, "gb2")

        # ---- PE warmup: trip the HAM clock gate while DMAs fill ----
        for i in range(N_WARMUP):
            wps = ps_proj.tile([128, 512], FP32, tag="proj_ps", name="warm_ps")
            nc.tensor.matmul(wps, lhsT=warm_sb[:, :128], rhs=warm_sb,
                             start=True, stop=True)

        qTp = singles.tile([128, DT, WPC], MM_DT, tag="qTp")
        xT = singles.tile([128, DT, WPC], MM_DT, tag="xT")

        def proj_T(w_sb, bias_sb, in_sb, out_sb, out_col0, n):
            for do in range(DT):
                ps = ps_proj.tile([128, 512], FP32, tag="proj_ps", name="proj_ps")
                ps = ps[:, :n]
                for ki in range(DT):
                    nc.tensor.matmul(
                        ps, lhsT=w_sb[:, ki, ts(do, 128)], rhs=in_sb[:, ki, :n],
                        start=(ki == 0), stop=(ki == DT - 1),
                    )
                nc.scalar.activation(
                    out=out_sb[:, do, ds(out_col0, n)], in_=ps, func=AF.Relu,
                    bias=bias_sb[:, do : do + 1], scale=1.0,
                )

        # ---- phase 1: q projection ----
        for sb in range(NSB):
            proj_T(wq_sb, bq_sb, q_in[sb], qTp, sb * 512, 512)

        # ---- phase 2: attention, software-pipelined ----
        residT = {}  # superblock -> tile [128, DT, 512]
        kTp = {}     # block -> k-projection tile
        W_sb = {}    # block -> masked scoresT (the banded weight matrix)
        av4 = {}     # superblock -> [128, DT, 512] aggregated v (4 blocks)
        sr4 = {}     # superblock -> [1, 512] colsums of W (4 blocks)

        def emit_kproj(b):
            k_t, _ = kv_tiles[b]
            kp = ktp_p.tile([128, DT, 512], MM_DT, tag="kTp", name="kTp")
            proj_T(wk_sb, bk_sb, k_t, kp, 0, 512)
            kTp[b] = kp

        def emit_scores(b):
            # scT[k, w] = sum_d kTp[d, k] * qTp[d, w] for this block's keys
            sc_ps = ps_sc.tile([128, KC, 128], FP32, tag="sc_ps", name="sc_ps")
            for kc in range(KC):
                for ki in range(DT):
                    nc.tensor.matmul(
                        sc_ps[:, kc, :],
                        lhsT=kTp[b][:, ki, ts(kc, 128)],
                        rhs=qTp[:, ki, ts(b, 128)],
                        start=(ki == 0), stop=(ki == DT - 1),
                    )
            del kTp[b]
            # band mask -> sparse weight matrix W (bf16, zero off-band)
            w_t = w_p.tile([128, KC, 128], MM_DT, tag="W", name="W")
            nc.vector.tensor_tensor(w_t[:], sc_ps[:], maskT[:], op=OP.mult)
            W_sb[b] = w_t

        def emit_vagg(b):
            sb, col = b // 4, (b % 4) * 128
            _, v_t = kv_tiles.pop(b)
            w_t = W_sb[b]
            av_ps = ps_av.tile([128, DT, 128], FP32, tag="av_ps", name="av_ps")
            for dc in range(DT):
                for kc in range(KC):
                    nc.tensor.matmul(
                        av_ps[:, dc, :],
                        lhsT=v_t[:, kc, ts(dc, 128)],
                        rhs=w_t[:, kc, :],
                        start=(kc == 0), stop=(kc == KC - 1),
                    )
            if use_vbias:
                # srow[w] = sum_k W[k, w]  (for the bias term)
                sr_ps = ps_misc.tile([1, 128], FP32, tag="sr_ps", name="sr_ps", bufs=1)
                for kc in range(KC):
                    nc.tensor.matmul(
                        sr_ps, lhsT=ones_colb, rhs=w_t[:, kc, :],
                        start=(kc == 0), stop=(kc == KC - 1),
                    )
            if col == 0:
                av4[sb] = av_p.tile([128, DT, 512], MM_DT, tag="av4", name="av4")
                if use_vbias:
                    sr4[sb] = small.tile([1, 512], MM_DT, tag="sr4", name="sr4", bufs=2)
            nc.scalar.activation(
                out=av4[sb][:, :, ds(col, 128)], in_=av_ps[:], func=AF.Copy, scale=1.0)
            if use_vbias:
                nc.scalar.activation(
                    out=sr4[sb][:, ds(col, 128)], in_=sr_ps, func=AF.Copy, scale=1.0)
            del W_sb[b]

        def emit_aoproj_sb(sb):
            # ao projection for a whole superblock at N=512: the wv lhsT does
            # not depend on the block-diagonal attention structure.
            residT[sb] = resid_p.tile([128, DT, 512], MM_DT, tag="residT", name="residT")
            for do in range(DT):
                ao_ps = ps_ao.tile([128, 512], FP32, tag="ao_ps", name="ao_ps")
                for ki in range(DT):
                    nc.tensor.matmul(
                        ao_ps, lhsT=wv_sb[:, ki, ts(do, 128)],
                        rhs=av4[sb][:, ki, :],
                        start=(ki == 0), stop=(ki == DT - 1) and not use_vbias,
                    )
                if use_vbias:
                    nc.tensor.matmul(
                        ao_ps, lhsT=bv_row[:, ts(do, 128)], rhs=sr4[sb],
                        start=False, stop=True,
                    )
                nc.vector.tensor_tensor(
                    residT[sb][:, do, :], ao_ps, qTp[:, do, ts(sb, 512)], op=OP.add,
                )

        def emit_ln(resid_t, g_sb, gb_sb, out_cb, out_dt_chunked=False, affine=True):
            """Transposed LayerNorm over D for one 512-token superblock.

            resid_t: [128, DT, 512] fp16. out_cb(dt) -> output AP for d-tile dt.
            Stats run as fp16/bf16 ones-matmuls (1 cycle/row), broadcasts as
            fp16 rank-1 matmuls; PSUM accumulation is fp32 throughout.
            """
            sq = hT_p.tile([128, DT, 512], MM_DT, tag="sq", name="sq")
            nc.vector.tensor_tensor(sq[:], resid_t[:], resid_t[:], op=OP.mult)
            st_sum = ps_misc.tile([1, 512], FP32, tag="st", name="st_sum")
            for dt in range(DT):
                nc.tensor.matmul(
                    st_sum, lhsT=ones_colb, rhs=resid_t[:, dt, :],
                    start=(dt == 0), stop=(dt == DT - 1),
                )
            st_sq = ps_misc.tile([1, 512], FP32, tag="st", name="st_sq")
            for dt in range(DT):
                nc.tensor.matmul(
                    st_sq, lhsT=ones_colb, rhs=sq[:, dt, :],
                    start=(dt == 0), stop=(dt == DT - 1),
                )
            # Short scalar chain: varD = S2 + D*eps - S1^2/D = D*(var+eps);
            # rstd = sqrt(D * (1/varD)).
            mean = small.tile([1, 512], MM_DT, tag="mean", name="mean")
            nc.scalar.activation(out=mean, in_=st_sum, func=AF.Copy, scale=1.0 / D)
            m2d = small.tile([1, 512], FP32, tag="m2d", name="m2d")
            nc.vector.scalar_tensor_tensor(
                out=m2d, in0=mean, scalar=float(D), in1=mean,
                op0=OP.mult, op1=OP.mult,
            )
            varD = small.tile([1, 512], FP32, tag="varD", name="varD")
            nc.vector.scalar_tensor_tensor(
                out=varD, in0=st_sq, scalar=float(D) * EPS, in1=m2d,
                op0=OP.add, op1=OP.subtract,
            )
            r0 = small.tile([1, 512], FP32, tag="r0", name="r0")
            nc.vector.reciprocal_approx_fast(out=r0, in_=varD)
            rstd = small.tile([1, 512], FP16, tag="rstd", name="rstd")
            nc.scalar.activation(out=rstd, in_=r0, func=AF.Sqrt, scale=float(D))

            # bc tiles share the "st" tag/banks: st_sum/st_sq are consumed by
            # the small-ops above before these are written.
            bcm = ps_misc.tile([128, 512], FP32, tag="st", name="bcm")
            nc.tensor.matmul(bcm, lhsT=ones_rowb, rhs=mean, start=True, stop=True)
            bcr = ps_misc.tile([128, 512], FP32, tag="st", name="bcr")
            nc.tensor.matmul(bcr, lhsT=ones_rowh, rhs=rstd, start=True, stop=True)

            # t = (resid - mu) * rstd (fp16 scratch; centered so quantization
            # is relative to the normalized scale); out = t*g + b (ACT),
            # or written directly by the DVE mult when g==1, b==0.
            tmp = hT_p.tile([128, DT, 512], FP16, tag="tscr", name="tscr")
            for dt in range(DT):
                nc.vector.tensor_tensor(tmp[:, dt, :], resid_t[:, dt, :], bcm, op=OP.subtract)
                if affine:
                    nc.vector.tensor_tensor(tmp[:, dt, :], tmp[:, dt, :], bcr, op=OP.mult)
                    nc.scalar.activation(
                        out=out_cb(dt), in_=tmp[:, dt, :], func=AF.Identity,
                        bias=gb_sb[:, dt : dt + 1], scale=g_sb[:, dt : dt + 1],
                    )
                else:
                    nc.vector.tensor_tensor(out_cb(dt), tmp[:, dt, :], bcr, op=OP.mult)
                if out_dt_chunked:
                    out_dt_chunked(dt)

        def emit_ln1(sb):
            emit_ln(residT[sb], g1_sb, gb1_sb,
                    lambda dt: xT[:, dt, ts(sb, 512)], affine=affine1)

        def emit_ffn1(sb):
            hT = hT_p.tile([128, DT, 512], MM_DT, tag="hT", name="hT")
            for ht in range(DT):
                ps = ps_proj.tile([128, 512], FP32, tag="proj_ps", name="ffn1_ps")
                for ki in range(DT):
                    nc.tensor.matmul(
                        ps, lhsT=late["w1"][:, ki, ts(ht, 128)], rhs=xT[:, ki, ts(sb, 512)],
                        start=(ki == 0), stop=(ki == DT - 1),
                    )
                nc.scalar.activation(
                    out=hT[:, ht, :], in_=ps, func=AF.Relu,
                    bias=late["b1"][:, ht : ht + 1], scale=1.0,
                )
            return hT

        def emit_ffn2(sb, hT):
            resid2 = resid_p.tile([128, DT, 512], MM_DT, tag="resid2", name="resid2")
            for dt in range(DT):
                ps = ps_proj.tile([128, 512], FP32, tag="proj_ps", name="ffn2_ps")
                for hi in range(DT):
                    nc.tensor.matmul(
                        ps, lhsT=late["w2"][:, hi, ts(dt, 128)], rhs=hT[:, hi, :],
                        start=(hi == 0), stop=(hi == DT - 1),
                    )
                nc.vector.scalar_tensor_tensor(
                    out=resid2[:, dt, :], in0=ps, scalar=late["b2"][:, dt : dt + 1],
                    in1=xT[:, dt, ts(sb, 512)], op0=OP.add, op1=OP.add,
                )
            return resid2

        def emit_ln2_out(sb, resid2):
            out_sb = out_p.tile([128, DT, 512], FP32, tag="out_sb", name="out_sb")

            def dma_dt(dt):
                nc.sync.dma_start(
                    out=outT_t[:, dt, ts(sb, 512)], in_=out_sb[:, dt, :]
                )

            emit_ln(resid2, late["g2"], late["gb2"],
                    lambda dt: out_sb[:, dt, :], out_dt_chunked=dma_dt,
                    affine=affine2)

        # pipeline: k-proj(b), scoresT(b-1), v-agg(b-2); ao projection and
        # LN1 fire once per superblock when its 4 blocks' v-agg is emitted.
        for b in range(NBLK + 2):
            if b < NBLK:
                emit_kproj(b)
            if b == 0:
                prefetch_kv(2)
                load_late_consts()
            elif b < NBLK:
                prefetch_kv(b + 2)
            if 1 <= b <= NBLK:
                emit_scores(b - 1)
            if 2 <= b <= NBLK + 1:
                emit_vagg(b - 2)
            if b - 2 == 3:  # v-agg(0..3) emitted -> superblock 0 ready
                emit_aoproj_sb(0)
                emit_ln1(0)

        emit_aoproj_sb(1)
        emit_ln1(NSB - 1)
        hT0 = emit_ffn1(0)
        r20 = emit_ffn2(0, hT0)
        hT1 = emit_ffn1(1)
        emit_ln2_out(0, r20)
        r21 = emit_ffn2(1, hT1)
        emit_ln2_out(1, r21)

    nc.finalize()
    return nc


def kernel(**inputs):
    # Specialize on actually-zero biases / identity LN affines (checked at
    # runtime; the general program is built when they are nontrivial).
    use_vbias = bool(np.any(np.asarray(inputs["b_v"], dtype=np.float32)))
    affine1 = not (
        np.all(np.asarray(inputs["ln1_g"], dtype=np.float32) == 1.0)
        and not np.any(np.asarray(inputs["ln1_b"], dtype=np.float32))
    )
    affine2 = not (
        np.all(np.asarray(inputs["ln2_g"], dtype=np.float32) == 1.0)
        and not np.any(np.asarray(inputs["ln2_b"], dtype=np.float32))
    )
    pkey = ("prog", use_vbias, affine1, affine2)
    if pkey not in _CACHE:
        _CACHE[pkey] = build_program(use_vbias, affine1, affine2)
    nc = _CACHE[pkey]

    import ml_dtypes

    f32 = lambda x: np.ascontiguousarray(np.asarray(x), dtype=np.float32)
    bf16 = lambda x: np.ascontiguousarray(np.asarray(x, dtype=np.float32).astype(ml_dtypes.bfloat16))
    query, key_, value = f32(inputs["query"]), f32(inputs["key"]), f32(inputs["value"])

    shared = {
        n: f32(inputs[n])
        for n in ("b_q", "b_k", "ffn_b1", "ffn_b2",
                  "ln1_g", "ln1_b", "ln2_g", "ln2_b")
    }
    for n in ("w_q", "w_k", "w_v", "ffn_w1", "ffn_w2"):
        shared[n] = bf16(inputs[n])
    shared["bv_row"] = bf16(np.asarray(inputs["b_v"], dtype=np.float32).reshape(1, D))
    # maskT[p, kc, w] = 1 where key (kc*128+p) belongs to window w of the block
    p_idx = np.arange(128)[:, None, None]
    kc_idx = np.arange(KC)[None, :, None]
    w_idx = np.arange(128)[None, None, :]
    shared["maskT"] = (w_idx == kc_idx * 32 + p_idx // 4).astype(np.float32)

    in_maps = []
    for c in range(NCORES):
        bi, half = c // 2, c % 2
        w0 = half * WPC
        m = dict(shared)
        m["qT"] = bf16(query[bi, w0 : w0 + WPC, :].T)
        m["kT"] = bf16(key_[bi, w0 * F : (w0 + WPC) * F, :].T)
        m["vN"] = bf16(value[bi, w0 * F : (w0 + WPC) * F, :])
        in_maps.append(m)

    res = run_bass_kernel_spmd(nc, in_maps, core_ids=list(range(NCORES)))
    _CACHE["last_result"] = res
    out = np.empty((B, SQ, D), dtype=np.float32)
    for c in range(NCORES):
        bi, half = c // 2, c % 2
        w0 = half * WPC
        out[bi, w0 : w0 + WPC, :] = res.results[c]["outT"].T
    return out
